# revision 11
# baseline (speedup 1.0000x reference)
"""MoE (DeepSeek-style) routed+shared expert forward on 8 TRN2 NeuronCores.

Strategy (expert-parallel, host-side dispatch):
  - Host computes the gate (softmax + top-2) in float64 and gathers each
    expert's routed tokens (padded to a uniform capacity C).
  - Core e runs expert e's routed tokens through the SwiGLU FFN in
    fp8-e4m3 with DoubleRow matmuls (2 fp8 MACs per PE per cycle), plus
    a 1/8 slice of all tokens through the replicated shared-expert MLP
    in bf16 (the shared output dominates the result norm, so it needs
    the extra mantissa; the routed output is small enough for fp8).
  - Weights are pre-scaled on the host (w1*64, w3*16, w2*64) to center
    them in e4m3's normal range; the scales unwind for free inside the
    scalar-engine activation (out = f(in*scale + bias)).
  - All weights (routed fp8 + shared bf16, ~150KB/partition) are SBUF
    resident simultaneously; the shared-expert weight DMAs trickle in
    between routed blocks so the PE never idles at the phase boundary
    (idle >3us drops the HAM clock to 4/8 for ~27us).
  - Activations/weights are fed transposed (features on partitions,
    tokens on the free dim) so w1/w3 -> swiglu -> w2 needs no on-chip
    transposes.  Outputs return as bf16; host upcasts, scales by the
    gate weights and scatters back by routing index.
"""

import sys

if "/opt/trn_rl_repo" not in sys.path:
    sys.path.insert(0, "/opt/trn_rl_repo")

import ml_dtypes
import numpy as np

import concourse.bass as bass
import concourse.tile as tile
from concourse import bacc, mybir
from concourse import bass_utils

B, S, DIM = 4, 2048, 1024
T = B * S
INTER = 1024
E = 8
TOPK = 2
ROUTE_SCALE = 1.0
SHARED_INTER = 2048
N_CORES = 8
TOKS_SHARED = T // N_CORES  # shared-expert tokens per core

S1, S3, S2 = 64.0, 16.0, 64.0  # host-side fp8 weight scales

F32 = mybir.dt.float32
F8 = mybir.dt.float8e4
BF16 = mybir.dt.bfloat16
SILU = mybir.ActivationFunctionType.Silu
IDENT = mybir.ActivationFunctionType.Identity
DR = mybir.MatmulPerfMode.DoubleRow

_program_cache = {}


def _capacity(c0):
    """Pick (nblk, N): token blocks of width N (multiple of 16, <=256 so
    the DoubleRow moving AP stays within the 512 free-dim limit)."""
    c0 = max(c0, 256)
    nblk = -(-c0 // 256)
    n = -(-c0 // nblk)
    n = (n + 15) & ~15
    return nblk, n


def build_program(nblk, ntok, zero_b3):
    """Per-core SPMD Bass program.

    Phase 1 (routed expert): fp8 DoubleRow matmuls, all weights resident.
    Phase 2 (shared expert): bf16 matmuls.  Shared weights are DMA'd in
    slices interleaved with the phase-1 block loop.
    """
    C = nblk * ntok
    nc = bacc.Bacc("TRN2", target_bir_lowering=False, debug=False,
                   num_devices=N_CORES)

    def din(name, shape, dt):
        return nc.dram_tensor(name, shape, dt, kind="ExternalInput").ap()

    xe = din("xe", (DIM, C), F8)              # routed tokens, transposed
    xs = din("xs", (DIM, TOKS_SHARED), BF16)  # shared-token slice, transposed
    w1t = din("w1t", (DIM, INTER), F8)        # (w1[e]*S1).T
    w3t = din("w3t", (DIM, INTER), F8)        # (w3[e]*S3).T
    w2t = din("w2t", (INTER, DIM), F8)        # (w2[e]*S2).T
    ws1t = din("ws1t", (DIM, SHARED_INTER), BF16)
    ws3t = din("ws3t", (DIM, SHARED_INTER), BF16)
    ws2t = din("ws2t", (SHARED_INTER, DIM), BF16)
    biases = din("biases", (128, 64), F32)    # host-packed per-partition
    ye = nc.dram_tensor("ye", (DIM, C), BF16, kind="ExternalOutput").ap()
    ys = nc.dram_tensor("ys", (DIM, TOKS_SHARED), BF16,
                        kind="ExternalOutput").ap()

    ND = DIM // 128           # 8 k-tiles over DIM (4 DoubleRow pairs)
    NI = INTER // 128         # 8 tiles over INTER
    NS = SHARED_INTER // 128  # 16 tiles over SHARED_INTER
    NP = ND // 2              # 4 DoubleRow k-pairs

    xe_r = xe.rearrange("(dk p) c -> p dk c", p=128)
    xs_r = xs.rearrange("(dk p) c -> p dk c", p=128)
    w1_r = w1t.rearrange("(dk p) i -> p dk i", p=128)
    w3_r = w3t.rearrange("(dk p) i -> p dk i", p=128)
    w2_r = w2t.rearrange("(mi p) d -> p mi d", p=128)
    ws1_r = ws1t.rearrange("(dk p) i -> p dk i", p=128)
    ws3_r = ws3t.rearrange("(dk p) i -> p dk i", p=128)
    ws2_r = ws2t.rearrange("(mi p) d -> p mi d", p=128)
    ye_r = ye.rearrange("(md p) c -> p md c", p=128)
    ys_r = ys.rearrange("(md p) c -> p md c", p=128)

    with tile.TileContext(nc) as tc:
        with tc.tile_pool(name="bias", bufs=1) as bpool, \
             tc.tile_pool(name="wexp", bufs=1) as wpool, \
             tc.tile_pool(name="wsh", bufs=1, side="right") as wspool, \
             tc.tile_pool(name="xsp", bufs=1, side="right") as xsp, \
             tc.tile_pool(name="xbp", bufs=3) as xpool, \
             tc.tile_pool(name="hbp", bufs=3) as hpool, \
             tc.tile_pool(name="hsp", bufs=2) as hspool, \
             tc.tile_pool(name="tmp", bufs=3) as tpool, \
             tc.tile_pool(name="yout", bufs=3) as ypool, \
             tc.tile_pool(name="psA", bufs=3, space="PSUM") as pspool, \
             tc.tile_pool(name="psY", bufs=2, space="PSUM") as pypool:
            ball = bpool.tile([128, 64], F32, tag="biases")
            nc.sync.dma_start(ball[:], biases[:])
            b1_sb = ball[:, 0:NI]
            b2_sb = ball[:, NI:NI + ND]
            b3_sb = ball[:, 16:16 + NI]          # b3*S3 (non-zero path)
            bs1_sb = ball[:, 24:24 + NS]
            bs3_sb = ball[:, 40:40 + NS]
            bs2_sb = ball[:, 56:56 + ND]

            # ---- routed weights (fp8, one consolidated tile each);
            # w1 + first tokens first so the PE starts early ----
            w1_sb = wpool.tile([128, ND, INTER], F8, tag="w1", name="w1")
            nc.sync.dma_start(w1_sb[:, 0:ND // 2, :], w1_r[:, 0:ND // 2, :])

            # ---- deferred loads: shared weights + xs, trickled in as a
            # few large chunks (each DMA trigger costs ~0.7us of sync-
            # engine time, so fewer, bigger transfers) ----
            ws1_sb = wspool.tile([128, ND, SHARED_INTER], BF16, tag="ws1",
                                 name="ws1")
            ws3_sb = wspool.tile([128, ND, SHARED_INTER], BF16, tag="ws3",
                                 name="ws3")
            ws2_sb = wspool.tile([128, NS, DIM], BF16, tag="ws2", name="ws2")
            xs_sb = xsp.tile([128, ND, TOKS_SHARED], BF16, tag="xs",
                             name="xs")
            deferred = []
            for a in range(0, ND, 2):       # xs: 1MB chunks
                deferred.append((xs_sb[:, a:a + 2, :], xs_r[:, a:a + 2, :]))
            for a in range(0, ND, 2):       # ws1/ws3: 1MB chunks
                deferred.append((ws1_sb[:, a:a + 2, :], ws1_r[:, a:a + 2, :]))
                deferred.append((ws3_sb[:, a:a + 2, :], ws3_r[:, a:a + 2, :]))
            for a in range(0, NS, 4):       # ws2: 1MB chunks
                deferred.append((ws2_sb[:, a:a + 4, :], ws2_r[:, a:a + 4, :]))

            def load_xb(off):
                t = xpool.tile([128, ND, ntok], F8, tag="xb", name="xb")
                nc.sync.dma_start(t[:], xe_r[:, :, off:off + ntok])
                return t

            xb_next = load_xb(0)
            nc.sync.dma_start(w1_sb[:, ND // 2:, :], w1_r[:, ND // 2:, :])
            w3_sb = wpool.tile([128, ND, INTER], F8, tag="w3", name="w3")
            nc.sync.dma_start(w3_sb[:], w3_r[:])
            w2_sb = wpool.tile([128, NI, DIM], F8, tag="w2", name="w2")
            nc.sync.dma_start(w2_sb[:], w2_r[:])

            # ---------- Phase 1: routed expert (fp8 DoubleRow) ----------
            # Software-pipelined: mi_stage(block i+1) is emitted before
            # md_stage(block i), so the PE->scalar->vector chain that
            # produces hb has a full stage of slack before the w2 matmuls
            # consume it (otherwise the PE stalls ~2-4us per block and the
            # HAM clock drops to 4/8).
            def mi_stage(xb):
                hb = hpool.tile([128, ND, ntok], F8, tag="hb", name="hb")
                for mi in range(NI):
                    ps1 = pspool.tile([128, ntok], F32, tag="ps1",
                                      padded_shape=[128, 512])
                    ps3 = pspool.tile([128, ntok], F32, tag="ps3",
                                      padded_shape=[128, 512])
                    for j in range(NP):
                        nc.tensor.matmul(
                            ps1[:], w1_sb[:, 2 * j:2 * j + 2,
                                          mi * 128:(mi + 1) * 128],
                            xb[:, 2 * j:2 * j + 2, :],
                            start=(j == 0), stop=(j == NP - 1),
                            perf_mode=DR)
                    for j in range(NP):
                        nc.tensor.matmul(
                            ps3[:], w3_sb[:, 2 * j:2 * j + 2,
                                          mi * 128:(mi + 1) * 128],
                            xb[:, 2 * j:2 * j + 2, :],
                            start=(j == 0), stop=(j == NP - 1),
                            perf_mode=DR)
                    t1 = tpool.tile([128, ntok], BF16, tag="t1", name="t1")
                    nc.scalar.activation(t1[:], ps1[:], SILU,
                                         bias=b1_sb[:, mi:mi + 1],
                                         scale=1.0 / S1)
                    hslot = hb[:, mi, :]
                    if zero_b3:
                        nc.vector.tensor_mul(hslot, t1[:], ps3[:])
                    else:
                        t3 = tpool.tile([128, ntok], F32, tag="t3", name="t3")
                        nc.scalar.activation(t3[:], ps3[:], IDENT,
                                             bias=b3_sb[:, mi:mi + 1])
                        nc.vector.tensor_mul(hslot, t1[:], t3[:])
                return hb

            def md_stage(hb, off):
                yst = ypool.tile([128, ND, ntok], BF16, tag="yt", name="yt")
                for md in range(ND):
                    psy = pypool.tile([128, ntok], F32, tag="psy",
                                      padded_shape=[128, 512])
                    for j in range(NP):
                        nc.tensor.matmul(
                            psy[:], w2_sb[:, 2 * j:2 * j + 2,
                                          md * 128:(md + 1) * 128],
                            hb[:, 2 * j:2 * j + 2, :],
                            start=(j == 0), stop=(j == NP - 1),
                            perf_mode=DR)
                    nc.scalar.activation(yst[:, md, :], psy[:], IDENT,
                                         bias=b2_sb[:, md:md + 1],
                                         scale=1.0 / (S3 * S2))
                nc.sync.dma_start(ye_r[:, :, off:off + ntok], yst[:])

            hb_prev = None
            for bi in range(nblk):
                xb = xb_next
                if bi + 1 < nblk:
                    xb_next = load_xb((bi + 1) * ntok)
                hb = mi_stage(xb)
                if hb_prev is not None:
                    md_stage(hb_prev, (bi - 1) * ntok)
                hb_prev = hb
                # bulk shared-weight chunks go last so the latency-critical
                # xb prefetch and ye store precede them on the sync queue
                for _ in range(2):
                    if deferred:
                        dt, dsrc = deferred.pop(0)
                        nc.sync.dma_start(dt, dsrc)
            md_stage(hb_prev, (nblk - 1) * ntok)

            # ---------- Phase 2: shared expert (bf16), same pipelining ----
            def mi_stage_s(off, n):
                hs = hspool.tile([128, NS, n], BF16, tag="hs", name="hs")
                for mi in range(NS):
                    ps1 = pspool.tile([128, n], F32, tag="ps1",
                                      padded_shape=[128, 512])
                    ps3 = pspool.tile([128, n], F32, tag="ps3",
                                      padded_shape=[128, 512])
                    for dk in range(ND):
                        nc.tensor.matmul(
                            ps1[:], ws1_sb[:, dk, mi * 128:(mi + 1) * 128],
                            xs_sb[:, dk, off:off + n],
                            start=(dk == 0), stop=(dk == ND - 1))
                    for dk in range(ND):
                        nc.tensor.matmul(
                            ps3[:], ws3_sb[:, dk, mi * 128:(mi + 1) * 128],
                            xs_sb[:, dk, off:off + n],
                            start=(dk == 0), stop=(dk == ND - 1))
                    t1 = tpool.tile([128, n], BF16, tag="t1s", name="t1s")
                    nc.scalar.activation(t1[:], ps1[:], SILU,
                                         bias=bs1_sb[:, mi:mi + 1])
                    if zero_b3:
                        nc.vector.tensor_mul(hs[:, mi, :], t1[:], ps3[:])
                    else:
                        t3 = tpool.tile([128, n], F32, tag="t3s", name="t3s")
                        nc.scalar.activation(t3[:], ps3[:], IDENT,
                                             bias=bs3_sb[:, mi:mi + 1])
                        nc.vector.tensor_mul(hs[:, mi, :], t1[:], t3[:])
                return hs

            def md_stage_s(hs, off, n):
                for md in range(ND):
                    psy = pypool.tile([128, n], F32, tag="psy",
                                      padded_shape=[128, 512])
                    for mi in range(NS):
                        nc.tensor.matmul(
                            psy[:], ws2_sb[:, mi, md * 128:(md + 1) * 128],
                            hs[:, mi, :], start=(mi == 0), stop=(mi == NS - 1))
                    yt = ypool.tile([128, n], BF16, tag="yts", name="yts")
                    nc.scalar.activation(yt[:], psy[:], IDENT,
                                         bias=bs2_sb[:, md:md + 1])
                    nc.sync.dma_start(ys_r[:, md, off:off + n], yt[:])

            sblocks = [(off, min(512, TOKS_SHARED - off))
                       for off in range(0, TOKS_SHARED, 512)]
            hs_prev = None
            for si, (off, n) in enumerate(sblocks):
                hs = mi_stage_s(off, n)
                if hs_prev is not None:
                    md_stage_s(hs_prev, *sblocks[si - 1])
                hs_prev = hs
            md_stage_s(hs_prev, *sblocks[-1])

    nc.compile()
    return nc


def _pack_biases(b1, b3, b2, bs1, bs3, bs2):
    """Pack bias vectors into one [128, 64] per-partition table."""
    out = np.zeros((128, 64), np.float32)
    cols = [(b1, 0), (b2, 8), (b3 * S3, 16), (bs1, 24), (bs3, 40), (bs2, 56)]
    for vec, c0 in cols:
        k = len(vec) // 128
        out[:, c0:c0 + k] = vec.reshape(k, 128).T
    return out


def _gate_host(xt, gate_w, gate_b):
    """Softmax gate + top-2 routing, computed in float64 on the host."""
    logits = xt.astype(np.float64) @ gate_w.astype(np.float64).T \
        + gate_b.astype(np.float64)
    m = logits.max(axis=-1, keepdims=True)
    p = np.exp(logits - m)
    scores = p / p.sum(axis=-1, keepdims=True)
    order = np.argsort(-scores, axis=1, kind="stable")
    top_i = order[:, :TOPK]
    top_w = (np.take_along_axis(scores, top_i, axis=1)
             * ROUTE_SCALE).astype(np.float32)
    return top_i, top_w


def _q8(a, scale):
    return np.clip(a * scale, -240.0, 240.0).astype(ml_dtypes.float8_e4m3)


def run(inputs, trace=False):
    x = np.ascontiguousarray(np.asarray(inputs["x"], dtype=np.float32))
    gate_w = np.asarray(inputs["gate_w"], dtype=np.float32)
    gate_b = np.asarray(inputs["gate_b"], dtype=np.float32)
    w1 = np.asarray(inputs["w1"], dtype=np.float32)
    b1 = np.asarray(inputs["b1"], dtype=np.float32)
    w3 = np.asarray(inputs["w3"], dtype=np.float32)
    b3 = np.asarray(inputs["b3"], dtype=np.float32)
    w2 = np.asarray(inputs["w2"], dtype=np.float32)
    b2 = np.asarray(inputs["b2"], dtype=np.float32)
    ws1 = np.asarray(inputs["ws1"], dtype=np.float32)
    bs1 = np.asarray(inputs["bs1"], dtype=np.float32)
    ws3 = np.asarray(inputs["ws3"], dtype=np.float32)
    bs3 = np.asarray(inputs["bs3"], dtype=np.float32)
    ws2 = np.asarray(inputs["ws2"], dtype=np.float32)
    bs2 = np.asarray(inputs["bs2"], dtype=np.float32)

    xt = x.reshape(T, DIM)
    top_i, top_w = _gate_host(xt, gate_w, gate_b)

    # Dispatch: token lists + gate weights per expert.
    idx, wgt = [], []
    for e in range(E):
        toks = np.nonzero((top_i == e).any(axis=1))[0]
        idx.append(toks)
        slot = (top_i[toks] == e)            # [n_e, TOPK], one True per row
        wgt.append(top_w[toks][slot])

    nblk, ntok = _capacity(max(len(i) for i in idx))
    C = nblk * ntok
    zero_b3 = bool(np.all(b3 == 0) and np.all(bs3 == 0))

    bf16 = ml_dtypes.bfloat16
    ws1t = np.ascontiguousarray(ws1.T).astype(bf16)
    ws3t = np.ascontiguousarray(ws3.T).astype(bf16)
    ws2t = np.ascontiguousarray(ws2.T).astype(bf16)
    xt_bf = xt.astype(bf16)
    xq8 = np.ascontiguousarray(_q8(xt, 1.0).T)   # [DIM, T] fp8

    in_maps = []
    for e in range(E):
        xe = np.zeros((DIM, C), ml_dtypes.float8_e4m3)
        xe[:, :len(idx[e])] = xq8[:, idx[e]]
        sl = slice(TOKS_SHARED * e, TOKS_SHARED * (e + 1))
        in_maps.append({
            "xe": xe,
            "xs": np.ascontiguousarray(xt_bf[sl].T),
            "w1t": _q8(np.ascontiguousarray(w1[e].T), S1),
            "w3t": _q8(np.ascontiguousarray(w3[e].T), S3),
            "w2t": _q8(np.ascontiguousarray(w2[e].T), S2),
            "ws1t": ws1t, "ws3t": ws3t, "ws2t": ws2t,
            "biases": _pack_biases(b1[e], b3[e], b2[e], bs1, bs3, bs2),
        })

    key = (nblk, ntok, zero_b3)
    if key not in _program_cache:
        _program_cache[key] = build_program(nblk, ntok, zero_b3)
    nc = _program_cache[key]

    res = bass_utils.run_bass_kernel_spmd(
        nc, in_maps, core_ids=list(range(N_CORES)), trace=trace)

    y = np.empty((T, DIM), np.float32)
    for e in range(E):
        sl = slice(TOKS_SHARED * e, TOKS_SHARED * (e + 1))
        y[sl] = res.results[e]["ys"].astype(np.float32).T
    for e in range(E):
        ye = res.results[e]["ye"].astype(np.float32)
        y[idx[e]] += ye[:, :len(idx[e])].T * wgt[e][:, None]
    return y.reshape(B, S, DIM), res


def kernel(**inputs) -> np.ndarray:
    out, _ = run(inputs, trace=False)
    return out


# revision 12
# speedup vs baseline: 1.0010x; 1.0010x over previous
"""MoE (DeepSeek-style) routed+shared expert forward on 8 TRN2 NeuronCores.

Strategy (expert-parallel, host-side dispatch):
  - Host computes the gate (softmax + top-2) in float64 and gathers each
    expert's routed tokens (padded to a uniform capacity C).
  - Core e runs expert e's routed tokens through the SwiGLU FFN in
    fp8-e4m3 with DoubleRow matmuls (2 fp8 MACs per PE per cycle), plus
    a 1/8 slice of all tokens through the replicated shared-expert MLP
    in bf16 (the shared output dominates the result norm, so it needs
    the extra mantissa; the routed output is small enough for fp8).
  - Weights are pre-scaled on the host (w1*64, w3*16, w2*64) to center
    them in e4m3's normal range; the scales unwind for free inside the
    scalar-engine activation (out = f(in*scale + bias)).
  - All weights (routed fp8 + shared bf16, ~150KB/partition) are SBUF
    resident simultaneously; the shared-expert weight DMAs trickle in
    between routed blocks so the PE never idles at the phase boundary
    (idle >3us drops the HAM clock to 4/8 for ~27us).
  - Activations/weights are fed transposed (features on partitions,
    tokens on the free dim) so w1/w3 -> swiglu -> w2 needs no on-chip
    transposes.  Outputs return as bf16; host upcasts, scales by the
    gate weights and scatters back by routing index.
"""

import sys

if "/opt/trn_rl_repo" not in sys.path:
    sys.path.insert(0, "/opt/trn_rl_repo")

import ml_dtypes
import numpy as np

import concourse.bass as bass
import concourse.tile as tile
from concourse import bacc, mybir
from concourse import bass_utils

B, S, DIM = 4, 2048, 1024
T = B * S
INTER = 1024
E = 8
TOPK = 2
ROUTE_SCALE = 1.0
SHARED_INTER = 2048
N_CORES = 8
TOKS_SHARED = T // N_CORES  # shared-expert tokens per core

S1, S3, S2 = 64.0, 16.0, 64.0  # host-side fp8 weight scales

F32 = mybir.dt.float32
F8 = mybir.dt.float8e4
BF16 = mybir.dt.bfloat16
SILU = mybir.ActivationFunctionType.Silu
IDENT = mybir.ActivationFunctionType.Identity
DR = mybir.MatmulPerfMode.DoubleRow

_program_cache = {}


def _capacity(c0):
    """Pick (nblk, N): token blocks of width N (multiple of 16, <=256 so
    the DoubleRow moving AP stays within the 512 free-dim limit)."""
    c0 = max(c0, 256)
    nblk = -(-c0 // 256)
    n = -(-c0 // nblk)
    n = (n + 15) & ~15
    return nblk, n


def build_program(nblk, ntok, zero_b3):
    """Per-core SPMD Bass program.

    Phase 1 (routed expert): fp8 DoubleRow matmuls, all weights resident.
    Phase 2 (shared expert): bf16 matmuls.  Shared weights are DMA'd in
    slices interleaved with the phase-1 block loop.
    """
    C = nblk * ntok
    nc = bacc.Bacc("TRN2", target_bir_lowering=False, debug=False,
                   num_devices=N_CORES)

    def din(name, shape, dt):
        return nc.dram_tensor(name, shape, dt, kind="ExternalInput").ap()

    xe = din("xe", (DIM, C), F8)              # routed tokens, transposed
    xs = din("xs", (DIM, TOKS_SHARED), BF16)  # shared-token slice, transposed
    w1t = din("w1t", (DIM, INTER), F8)        # (w1[e]*S1).T
    w3t = din("w3t", (DIM, INTER), F8)        # (w3[e]*S3).T
    w2t = din("w2t", (INTER, DIM), F8)        # (w2[e]*S2).T
    ws1t = din("ws1t", (DIM, SHARED_INTER), BF16)
    ws3t = din("ws3t", (DIM, SHARED_INTER), BF16)
    ws2t = din("ws2t", (SHARED_INTER, DIM), BF16)
    biases = din("biases", (128, 64), F32)    # host-packed per-partition
    ye = nc.dram_tensor("ye", (DIM, C), BF16, kind="ExternalOutput").ap()
    ys = nc.dram_tensor("ys", (DIM, TOKS_SHARED), BF16,
                        kind="ExternalOutput").ap()

    ND = DIM // 128           # 8 k-tiles over DIM (4 DoubleRow pairs)
    NI = INTER // 128         # 8 tiles over INTER
    NS = SHARED_INTER // 128  # 16 tiles over SHARED_INTER
    NP = ND // 2              # 4 DoubleRow k-pairs

    xe_r = xe.rearrange("(dk p) c -> p dk c", p=128)
    xs_r = xs.rearrange("(dk p) c -> p dk c", p=128)
    w1_r = w1t.rearrange("(dk p) i -> p dk i", p=128)
    w3_r = w3t.rearrange("(dk p) i -> p dk i", p=128)
    w2_r = w2t.rearrange("(mi p) d -> p mi d", p=128)
    ws1_r = ws1t.rearrange("(dk p) i -> p dk i", p=128)
    ws3_r = ws3t.rearrange("(dk p) i -> p dk i", p=128)
    ws2_r = ws2t.rearrange("(mi p) d -> p mi d", p=128)
    ye_r = ye.rearrange("(md p) c -> p md c", p=128)
    ys_r = ys.rearrange("(md p) c -> p md c", p=128)

    with tile.TileContext(nc) as tc:
        with tc.tile_pool(name="bias", bufs=1) as bpool, \
             tc.tile_pool(name="wexp", bufs=1) as wpool, \
             tc.tile_pool(name="wsh", bufs=1, side="right") as wspool, \
             tc.tile_pool(name="xsp", bufs=1, side="right") as xsp, \
             tc.tile_pool(name="xbp", bufs=3) as xpool, \
             tc.tile_pool(name="hbp", bufs=3) as hpool, \
             tc.tile_pool(name="hsp", bufs=2) as hspool, \
             tc.tile_pool(name="tmp", bufs=3) as tpool, \
             tc.tile_pool(name="yout", bufs=3) as ypool, \
             tc.tile_pool(name="psA", bufs=3, space="PSUM") as pspool, \
             tc.tile_pool(name="psY", bufs=2, space="PSUM") as pypool:
            ball = bpool.tile([128, 64], F32, tag="biases")
            nc.sync.dma_start(ball[:], biases[:])
            b1_sb = ball[:, 0:NI]
            b2_sb = ball[:, NI:NI + ND]
            b3_sb = ball[:, 16:16 + NI]          # b3*S3 (non-zero path)
            bs1_sb = ball[:, 24:24 + NS]
            bs3_sb = ball[:, 40:40 + NS]
            bs2_sb = ball[:, 56:56 + ND]

            # ---- routed weights (fp8, one consolidated tile each);
            # w1 + first tokens first so the PE starts early ----
            w1_sb = wpool.tile([128, ND, INTER], F8, tag="w1", name="w1")
            nc.sync.dma_start(w1_sb[:, 0:ND // 2, :], w1_r[:, 0:ND // 2, :])

            # ---- deferred loads: shared weights + xs, trickled in as a
            # few large chunks (each DMA trigger costs ~0.7us of sync-
            # engine time, so fewer, bigger transfers) ----
            ws1_sb = wspool.tile([128, ND, SHARED_INTER], BF16, tag="ws1",
                                 name="ws1")
            ws3_sb = wspool.tile([128, ND, SHARED_INTER], BF16, tag="ws3",
                                 name="ws3")
            ws2_sb = wspool.tile([128, NS, DIM], BF16, tag="ws2", name="ws2")
            xs_sb = xsp.tile([128, ND, TOKS_SHARED], BF16, tag="xs",
                             name="xs")
            deferred = []
            for a in range(0, ND, 2):       # xs: 1MB chunks
                deferred.append((xs_sb[:, a:a + 2, :], xs_r[:, a:a + 2, :]))
            for a in range(0, ND, 2):       # ws1/ws3: 1MB chunks
                deferred.append((ws1_sb[:, a:a + 2, :], ws1_r[:, a:a + 2, :]))
                deferred.append((ws3_sb[:, a:a + 2, :], ws3_r[:, a:a + 2, :]))
            for a in range(0, NS, 4):       # ws2: 1MB chunks
                deferred.append((ws2_sb[:, a:a + 4, :], ws2_r[:, a:a + 4, :]))

            def load_xb(off):
                t = xpool.tile([128, ND, ntok], F8, tag="xb", name="xb")
                nc.sync.dma_start(t[:], xe_r[:, :, off:off + ntok])
                return t

            xb_next = load_xb(0)
            nc.sync.dma_start(w1_sb[:, ND // 2:, :], w1_r[:, ND // 2:, :])
            w3_sb = wpool.tile([128, ND, INTER], F8, tag="w3", name="w3")
            nc.sync.dma_start(w3_sb[:], w3_r[:])
            w2_sb = wpool.tile([128, NI, DIM], F8, tag="w2", name="w2")
            nc.sync.dma_start(w2_sb[:], w2_r[:])

            # ---------- Phase 1: routed expert (fp8 DoubleRow) ----------
            # Software-pipelined: mi_stage(block i+1) is emitted before
            # md_stage(block i), so the PE->scalar->vector chain that
            # produces hb has a full stage of slack before the w2 matmuls
            # consume it (otherwise the PE stalls ~2-4us per block and the
            # HAM clock drops to 4/8).
            def mi_stage(xb):
                hb = hpool.tile([128, ND, ntok], F8, tag="hb", name="hb")
                for mi in range(NI):
                    ps1 = pspool.tile([128, ntok], F32, tag="ps1",
                                      padded_shape=[128, 512])
                    ps3 = pspool.tile([128, ntok], F32, tag="ps3",
                                      padded_shape=[128, 512])
                    for j in range(NP):
                        nc.tensor.matmul(
                            ps1[:], w1_sb[:, 2 * j:2 * j + 2,
                                          mi * 128:(mi + 1) * 128],
                            xb[:, 2 * j:2 * j + 2, :],
                            start=(j == 0), stop=(j == NP - 1),
                            perf_mode=DR)
                    for j in range(NP):
                        nc.tensor.matmul(
                            ps3[:], w3_sb[:, 2 * j:2 * j + 2,
                                          mi * 128:(mi + 1) * 128],
                            xb[:, 2 * j:2 * j + 2, :],
                            start=(j == 0), stop=(j == NP - 1),
                            perf_mode=DR)
                    t1 = tpool.tile([128, ntok], BF16, tag="t1", name="t1")
                    nc.scalar.activation(t1[:], ps1[:], SILU,
                                         bias=b1_sb[:, mi:mi + 1],
                                         scale=1.0 / S1)
                    hslot = hb[:, mi, :]
                    if zero_b3:
                        nc.vector.tensor_mul(hslot, t1[:], ps3[:])
                    else:
                        t3 = tpool.tile([128, ntok], F32, tag="t3", name="t3")
                        nc.scalar.activation(t3[:], ps3[:], IDENT,
                                             bias=b3_sb[:, mi:mi + 1])
                        nc.vector.tensor_mul(hslot, t1[:], t3[:])
                return hb

            def md_stage(hb, off):
                yst = ypool.tile([128, ND, ntok], BF16, tag="yt", name="yt")
                for md in range(ND):
                    psy = pypool.tile([128, ntok], F32, tag="psy",
                                      padded_shape=[128, 512])
                    for j in range(NP):
                        nc.tensor.matmul(
                            psy[:], w2_sb[:, 2 * j:2 * j + 2,
                                          md * 128:(md + 1) * 128],
                            hb[:, 2 * j:2 * j + 2, :],
                            start=(j == 0), stop=(j == NP - 1),
                            perf_mode=DR)
                    # out = psy * (1/(S3*S2)) + b2 on the vector engine:
                    # the scalar engine's activation is slower than the 4
                    # matmuls feeding each psy and would throttle the PE
                    nc.vector.tensor_scalar(
                        yst[:, md, :], psy[:], 1.0 / (S3 * S2),
                        b2_sb[:, md:md + 1], op0=mybir.AluOpType.mult,
                        op1=mybir.AluOpType.add)
                nc.sync.dma_start(ye_r[:, :, off:off + ntok], yst[:])

            hb_prev = None
            for bi in range(nblk):
                xb = xb_next
                if bi + 1 < nblk:
                    xb_next = load_xb((bi + 1) * ntok)
                hb = mi_stage(xb)
                if hb_prev is not None:
                    md_stage(hb_prev, (bi - 1) * ntok)
                hb_prev = hb
                # bulk shared-weight chunks go last so the latency-critical
                # xb prefetch and ye store precede them on the sync queue
                for _ in range(2):
                    if deferred:
                        dt, dsrc = deferred.pop(0)
                        nc.sync.dma_start(dt, dsrc)
            md_stage(hb_prev, (nblk - 1) * ntok)

            # ---------- Phase 2: shared expert (bf16), same pipelining ----
            def mi_stage_s(off, n):
                hs = hspool.tile([128, NS, n], BF16, tag="hs", name="hs")
                for mi in range(NS):
                    ps1 = pspool.tile([128, n], F32, tag="ps1",
                                      padded_shape=[128, 512])
                    ps3 = pspool.tile([128, n], F32, tag="ps3",
                                      padded_shape=[128, 512])
                    for dk in range(ND):
                        nc.tensor.matmul(
                            ps1[:], ws1_sb[:, dk, mi * 128:(mi + 1) * 128],
                            xs_sb[:, dk, off:off + n],
                            start=(dk == 0), stop=(dk == ND - 1))
                    for dk in range(ND):
                        nc.tensor.matmul(
                            ps3[:], ws3_sb[:, dk, mi * 128:(mi + 1) * 128],
                            xs_sb[:, dk, off:off + n],
                            start=(dk == 0), stop=(dk == ND - 1))
                    t1 = tpool.tile([128, n], BF16, tag="t1s", name="t1s")
                    nc.scalar.activation(t1[:], ps1[:], SILU,
                                         bias=bs1_sb[:, mi:mi + 1])
                    if zero_b3:
                        nc.vector.tensor_mul(hs[:, mi, :], t1[:], ps3[:])
                    else:
                        t3 = tpool.tile([128, n], F32, tag="t3s", name="t3s")
                        nc.scalar.activation(t3[:], ps3[:], IDENT,
                                             bias=bs3_sb[:, mi:mi + 1])
                        nc.vector.tensor_mul(hs[:, mi, :], t1[:], t3[:])
                return hs

            def md_stage_s(hs, off, n):
                for md in range(ND):
                    psy = pypool.tile([128, n], F32, tag="psy",
                                      padded_shape=[128, 512])
                    for mi in range(NS):
                        nc.tensor.matmul(
                            psy[:], ws2_sb[:, mi, md * 128:(md + 1) * 128],
                            hs[:, mi, :], start=(mi == 0), stop=(mi == NS - 1))
                    yt = ypool.tile([128, n], BF16, tag="yts", name="yts")
                    nc.scalar.activation(yt[:], psy[:], IDENT,
                                         bias=bs2_sb[:, md:md + 1])
                    nc.sync.dma_start(ys_r[:, md, off:off + n], yt[:])

            sblocks = [(off, min(512, TOKS_SHARED - off))
                       for off in range(0, TOKS_SHARED, 512)]
            hs_prev = None
            for si, (off, n) in enumerate(sblocks):
                hs = mi_stage_s(off, n)
                if hs_prev is not None:
                    md_stage_s(hs_prev, *sblocks[si - 1])
                hs_prev = hs
            md_stage_s(hs_prev, *sblocks[-1])

    nc.compile()
    return nc


def _pack_biases(b1, b3, b2, bs1, bs3, bs2):
    """Pack bias vectors into one [128, 64] per-partition table."""
    out = np.zeros((128, 64), np.float32)
    cols = [(b1, 0), (b2, 8), (b3 * S3, 16), (bs1, 24), (bs3, 40), (bs2, 56)]
    for vec, c0 in cols:
        k = len(vec) // 128
        out[:, c0:c0 + k] = vec.reshape(k, 128).T
    return out


def _gate_host(xt, gate_w, gate_b):
    """Softmax gate + top-2 routing, computed in float64 on the host."""
    logits = xt.astype(np.float64) @ gate_w.astype(np.float64).T \
        + gate_b.astype(np.float64)
    m = logits.max(axis=-1, keepdims=True)
    p = np.exp(logits - m)
    scores = p / p.sum(axis=-1, keepdims=True)
    order = np.argsort(-scores, axis=1, kind="stable")
    top_i = order[:, :TOPK]
    top_w = (np.take_along_axis(scores, top_i, axis=1)
             * ROUTE_SCALE).astype(np.float32)
    return top_i, top_w


def _q8(a, scale):
    return np.clip(a * scale, -240.0, 240.0).astype(ml_dtypes.float8_e4m3)


def run(inputs, trace=False):
    x = np.ascontiguousarray(np.asarray(inputs["x"], dtype=np.float32))
    gate_w = np.asarray(inputs["gate_w"], dtype=np.float32)
    gate_b = np.asarray(inputs["gate_b"], dtype=np.float32)
    w1 = np.asarray(inputs["w1"], dtype=np.float32)
    b1 = np.asarray(inputs["b1"], dtype=np.float32)
    w3 = np.asarray(inputs["w3"], dtype=np.float32)
    b3 = np.asarray(inputs["b3"], dtype=np.float32)
    w2 = np.asarray(inputs["w2"], dtype=np.float32)
    b2 = np.asarray(inputs["b2"], dtype=np.float32)
    ws1 = np.asarray(inputs["ws1"], dtype=np.float32)
    bs1 = np.asarray(inputs["bs1"], dtype=np.float32)
    ws3 = np.asarray(inputs["ws3"], dtype=np.float32)
    bs3 = np.asarray(inputs["bs3"], dtype=np.float32)
    ws2 = np.asarray(inputs["ws2"], dtype=np.float32)
    bs2 = np.asarray(inputs["bs2"], dtype=np.float32)

    xt = x.reshape(T, DIM)
    top_i, top_w = _gate_host(xt, gate_w, gate_b)

    # Dispatch: token lists + gate weights per expert.
    idx, wgt = [], []
    for e in range(E):
        toks = np.nonzero((top_i == e).any(axis=1))[0]
        idx.append(toks)
        slot = (top_i[toks] == e)            # [n_e, TOPK], one True per row
        wgt.append(top_w[toks][slot])

    nblk, ntok = _capacity(max(len(i) for i in idx))
    C = nblk * ntok
    zero_b3 = bool(np.all(b3 == 0) and np.all(bs3 == 0))

    bf16 = ml_dtypes.bfloat16
    ws1t = np.ascontiguousarray(ws1.T).astype(bf16)
    ws3t = np.ascontiguousarray(ws3.T).astype(bf16)
    ws2t = np.ascontiguousarray(ws2.T).astype(bf16)
    xt_bf = xt.astype(bf16)
    xq8 = np.ascontiguousarray(_q8(xt, 1.0).T)   # [DIM, T] fp8

    in_maps = []
    for e in range(E):
        xe = np.zeros((DIM, C), ml_dtypes.float8_e4m3)
        xe[:, :len(idx[e])] = xq8[:, idx[e]]
        sl = slice(TOKS_SHARED * e, TOKS_SHARED * (e + 1))
        in_maps.append({
            "xe": xe,
            "xs": np.ascontiguousarray(xt_bf[sl].T),
            "w1t": _q8(np.ascontiguousarray(w1[e].T), S1),
            "w3t": _q8(np.ascontiguousarray(w3[e].T), S3),
            "w2t": _q8(np.ascontiguousarray(w2[e].T), S2),
            "ws1t": ws1t, "ws3t": ws3t, "ws2t": ws2t,
            "biases": _pack_biases(b1[e], b3[e], b2[e], bs1, bs3, bs2),
        })

    key = (nblk, ntok, zero_b3)
    if key not in _program_cache:
        _program_cache[key] = build_program(nblk, ntok, zero_b3)
    nc = _program_cache[key]

    res = bass_utils.run_bass_kernel_spmd(
        nc, in_maps, core_ids=list(range(N_CORES)), trace=trace)

    y = np.empty((T, DIM), np.float32)
    for e in range(E):
        sl = slice(TOKS_SHARED * e, TOKS_SHARED * (e + 1))
        y[sl] = res.results[e]["ys"].astype(np.float32).T
    for e in range(E):
        ye = res.results[e]["ye"].astype(np.float32)
        y[idx[e]] += ye[:, :len(idx[e])].T * wgt[e][:, None]
    return y.reshape(B, S, DIM), res


def kernel(**inputs) -> np.ndarray:
    out, _ = run(inputs, trace=False)
    return out
